# revision 15
# baseline (speedup 1.0000x reference)
"""BalanceLoss Trainium2 kernel.

Math restructuring (see reference _balance_loss):
  - pos_gt = (pos_sum >= B/2), neg_gt = (pos_sum < B/2) are always
    complementary, so every element is in exactly one of maj/min groups and
    maj_cnt/min_cnt = max/min(pos_sum, neg_sum): no 2nd reduction pass.
  - easy <=> (2t-1)*pred > ln2  <=>  (t==1 and softplus(-x) < ln1.5) or
    (t==0 and softplus(x) < ln1.5).
  - loss decomposes into 5 per-column sums over the batch:
        pos_sum = sum t
        S_B1 = sum_{t=1} softplus(-x)            (bce over positives)
        S_E1 = sum_{t=1, easy} softplus(-x)
        S_B0 = sum_{t=0} softplus(x)             (bce over negatives)
        S_E0 = sum_{t=0, easy} softplus(x)
    then per column:
        maj = positives if pos_gt else negatives
        loss_c = maj_scale*(S_Bmaj - S_Emaj) + min_scale*S_Bmin
        total = sum_c loss_c / (B*C)

On device (per core, data-parallel over batch), per chunk:
  ACT: ex = exp(x); sp = ln(ex+1) = softplus(x)    (no softplus LUT in build)
       tb = bf16 copy of t
  DVE: spn = sp - x; tspn = tb*spn; tm1 = tb-1; tnsp = tm1*sp  [= -(1-t)sp]
       relu1 = max(tspn-tau, 0)   m0 = min(tnsp+tau, 0)   (dual-op TS, 4x)
       ge1 = [tspn >= tau]        ge0 = [tnsp <= -tau]    (hard counts)
  PE:  ones[128,1]^T @ {tb,tspn,tnsp,relu1,m0,ge1,ge0} -> 7 PSUM rows
Host: S_E1 = S_B1 - R1 - tau*N1 with R1 = sum(relu1), R0 = -sum(m0), etc.
"""

import numpy as np

B_TOTAL = 131072
C = 128
N_CORES = 8
ROWS = B_TOTAL // N_CORES      # 16384 rows per core
FD = 2048                      # free-dim elements per chunk tile
J = FD // C                    # rows folded per partition per chunk
N_CHUNKS = ROWS * C // (128 * FD)   # 16
MM_N = 512                     # matmul moving free dim (one PSUM bank)
TAU = float(np.log(1.5))       # softplus(-ln2) boundary for "easy"

_CACHE = {}


def _pin_act_tables():
    """Force the single LUT set containing both exp and ln so the kernel
    loads one ACT table instead of ping-ponging between two (1.3us/reload).
    Set indices must keep matching act_info.json, so empty the others."""
    import concourse.bacc as bacc
    import concourse.hw_specs as hw_specs

    if getattr(hw_specs, "_act_tables_pinned", False):
        return
    orig = hw_specs.get_activation_tables

    def patched(arch):
        tabs = dict(orig(arch))
        keep = "natural_log_exp_and_others"
        if keep in tabs:
            tabs = {n: (s if n == keep else set()) for n, s in tabs.items()}
        return tabs

    hw_specs._act_tables_pinned = True
    hw_specs.get_activation_tables = patched
    bacc.get_activation_tables = patched


def _build_nc(dt_prod):
    import concourse.bacc as bacc
    import concourse.tile as tile
    from concourse import mybir

    _pin_act_tables()

    f32 = mybir.dt.float32
    AF = mybir.ActivationFunctionType
    OP = mybir.AluOpType

    nc = bacc.Bacc(None)
    pred = nc.dram_tensor("pred", [ROWS, C], f32, kind="ExternalInput")
    targ = nc.dram_tensor("targ", [ROWS, C], f32, kind="ExternalInput")
    out = nc.dram_tensor("partials", [1, 7 * MM_N], f32, kind="ExternalOutput")

    # row = m*FD + p*J + j ; per-partition contiguous J*C elements
    pred_r = pred.rearrange("(m p j) c -> m p (j c)", p=128, j=J)
    targ_r = targ.rearrange("(m p j) c -> m p (j c)", p=128, j=J)

    with tile.TileContext(nc) as tc:
        with (
            tc.tile_pool(name="singles", bufs=1) as singles,
            tc.tile_pool(name="io", bufs=3) as io,
            tc.tile_pool(name="work", bufs=2) as work,
            tc.tile_pool(name="psum", bufs=1, space="PSUM") as psum_pool,
        ):
            ones = singles.tile([128, 1], dt_prod)
            nc.vector.memset(ones, 1.0)
            acc = [
                psum_pool.tile([1, MM_N], f32, tag=f"acc{s}", name=f"acc{s}")
                for s in range(7)
            ]
            # Warmup matmul consumes the ones-memset dependency so that
            # steady-state matmuls carry at most one sync wait (walrus
            # LDWEIGHTS codegen supports only one).
            warm = psum_pool.tile([1, 1], f32, tag="warm")
            nc.tensor.matmul(warm, ones, ones, start=True, stop=True)
            n_sub = FD // MM_N
            for m in range(N_CHUNKS):
                x = io.tile([128, FD], f32, tag="x")
                t = io.tile([128, FD], f32, tag="t")
                nc.sync.dma_start(x, pred_r[m])
                nc.sync.dma_start(t, targ_r[m])

                # bf16 cast of t on ACT (ScalarE has spare capacity)
                tb = io.tile([128, FD], dt_prod, tag="tb")
                nc.scalar.copy(tb, t)

                # softplus via exp/ln (this compiler build has no softplus
                # ACT table): sp = ln(exp(x) + 1), kept in f32 so the
                # easy/hard boundary is decided at f32 precision.
                ex = work.tile([128, FD], mybir.dt.float32, tag="ex")
                sp = work.tile([128, FD], mybir.dt.float32, tag="sp")
                nc.scalar.activation(ex, x, AF.Exp)
                nc.scalar.activation(sp, ex, AF.Ln, bias=1.0)

                # tau-shifted streams: boundary sits at 0, so the bf16
                # outputs keep the exact f32 classification sign.
                spn2 = work.tile([128, FD], dt_prod, tag="spn2")
                tspn2 = work.tile([128, FD], dt_prod, tag="tspn2")
                tm1 = work.tile([128, FD], dt_prod, tag="tm1")
                tnsp2 = work.tile([128, FD], dt_prod, tag="tnsp2")
                relu1 = work.tile([128, FD], dt_prod, tag="relu1")
                m0 = work.tile([128, FD], dt_prod, tag="m0")
                ge1 = work.tile([128, FD], dt_prod, tag="ge1")
                ge0 = work.tile([128, FD], dt_prod, tag="ge0")
                # spn2 = (sp - tau) - x = softplus(-x) - tau
                nc.vector.scalar_tensor_tensor(
                    spn2, sp, TAU, x, OP.subtract, OP.subtract
                )
                nc.vector.tensor_tensor(tspn2, tb, spn2, OP.mult)
                nc.vector.tensor_scalar(tm1, tb, 1.0, None, OP.subtract)
                # tnsp2 = (sp - tau) * (t - 1)
                nc.vector.scalar_tensor_tensor(
                    tnsp2, sp, TAU, tm1, OP.subtract, OP.mult
                )
                # hard-sample clamps + counts (scalars 0.0 exact in bf16)
                nc.vector.tensor_scalar(relu1, tspn2, 0.0, None, OP.max)
                nc.vector.tensor_scalar(m0, tnsp2, 0.0, None, OP.min)
                nc.vector.tensor_scalar(ge1, tspn2, 0.0, None, OP.is_gt)
                nc.vector.tensor_scalar(ge0, tnsp2, 0.0, None, OP.is_lt)

                movers = [tb, tspn2, tnsp2, relu1, m0, ge1, ge0]
                for s, mv in enumerate(movers):
                    for jj in range(n_sub):
                        nc.tensor.matmul(
                            acc[s][:, :],
                            ones[:, :],
                            mv[:, jj * MM_N : (jj + 1) * MM_N],
                            start=(m == 0 and jj == 0),
                            stop=(m == N_CHUNKS - 1 and jj == n_sub - 1),
                        )

            res = singles.tile([1, 7 * MM_N], f32)
            for s in range(7):
                nc.scalar.copy(res[:, s * MM_N : (s + 1) * MM_N], acc[s][:, :])
            nc.sync.dma_start(out[:, :], res)
    nc.finalize()
    return nc


def _get_nc():
    if "nc" not in _CACHE:
        from concourse import mybir

        _CACHE["nc"] = _build_nc(mybir.dt.bfloat16)
    return _CACHE["nc"]


def _combine(parts):
    """parts: [n_cores, 7, MM_N] raw psum rows -> final scalar loss."""
    # psum column n sums rows with (j % (MM_N//C)) == n//C at col n % C,
    # summed over chunks/jj; fold the leftover j-groups and cores.
    S = parts.reshape(-1, 7, MM_N // C, C).sum(axis=(0, 2), dtype=np.float64)
    pos, t2, u2, r1, nr0, n1, n0 = S
    tau = float(np.float32(TAU))
    B = float(B_TOTAL)
    # t2 = sum t*(spn - tau) ; u2 = sum (t-1)*(sp - tau)
    sb1 = t2 + tau * pos
    sb0 = tau * (B - pos) - u2
    r0 = -nr0
    se1 = sb1 - r1 - tau * n1
    se0 = sb0 - r0 - tau * n0
    B = float(B_TOTAL)
    bal = 0.5 * B
    pos_gt = pos >= bal
    maj_cnt = np.where(pos_gt, pos, B - pos)
    min_cnt = B - maj_cnt
    maj_scale = bal / np.maximum(maj_cnt, 1.0)
    min_scale = np.where(min_cnt > 0, (B - bal) / np.maximum(min_cnt, 1.0), 1.0)
    s_maj_hard = np.where(pos_gt, sb1 - se1, sb0 - se0)
    s_min = np.where(pos_gt, sb0, sb1)
    total = (maj_scale * s_maj_hard + min_scale * s_min).sum()
    return np.float32(total / (B * C))


def kernel(pred: np.ndarray, target: np.ndarray) -> np.ndarray:
    from concourse.bass_utils import run_bass_kernel_spmd

    nc = _get_nc()
    pred = np.ascontiguousarray(pred, dtype=np.float32)
    target = np.ascontiguousarray(target, dtype=np.float32)
    in_maps = [
        {
            "pred": pred[i * ROWS : (i + 1) * ROWS],
            "targ": target[i * ROWS : (i + 1) * ROWS],
        }
        for i in range(N_CORES)
    ]
    res = run_bass_kernel_spmd(nc, in_maps, core_ids=list(range(N_CORES)))
    parts = np.stack([r["partials"].reshape(7, MM_N) for r in res.results])
    return _combine(parts)


# revision 16
# speedup vs baseline: 1.0612x; 1.0612x over previous
"""BalanceLoss Trainium2 kernel.

Math restructuring (see reference _balance_loss):
  - pos_gt = (pos_sum >= B/2), neg_gt = (pos_sum < B/2) are always
    complementary, so every element is in exactly one of maj/min groups and
    maj_cnt/min_cnt = max/min(pos_sum, neg_sum): no 2nd reduction pass.
  - easy <=> (2t-1)*pred > ln2  <=>  (t==1 and softplus(-x) < ln1.5) or
    (t==0 and softplus(x) < ln1.5).
  - loss decomposes into 5 per-column sums over the batch:
        pos_sum = sum t
        S_B1 = sum_{t=1} softplus(-x)            (bce over positives)
        S_E1 = sum_{t=1, easy} softplus(-x)
        S_B0 = sum_{t=0} softplus(x)             (bce over negatives)
        S_E0 = sum_{t=0, easy} softplus(x)
    then per column:
        maj = positives if pos_gt else negatives
        loss_c = maj_scale*(S_Bmaj - S_Emaj) + min_scale*S_Bmin
        total = sum_c loss_c / (B*C)

On device (per core, data-parallel over batch), per chunk:
  ACT: ex = exp(x); sp = ln(ex+1) = softplus(x)    (no softplus LUT in build)
       tb = bf16 copy of t
  DVE: spn = sp - x; tspn = tb*spn; tm1 = tb-1; tnsp = tm1*sp  [= -(1-t)sp]
       relu1 = max(tspn-tau, 0)   m0 = min(tnsp+tau, 0)   (dual-op TS, 4x)
       ge1 = [tspn >= tau]        ge0 = [tnsp <= -tau]    (hard counts)
  PE:  ones[128,1]^T @ {tb,tspn,tnsp,relu1,m0,ge1,ge0} -> 7 PSUM rows
Host: S_E1 = S_B1 - R1 - tau*N1 with R1 = sum(relu1), R0 = -sum(m0), etc.
"""

import numpy as np

B_TOTAL = 131072
C = 128
N_CORES = 8
ROWS = B_TOTAL // N_CORES      # 16384 rows per core
FD = 2048                      # free-dim elements per chunk tile
J = FD // C                    # rows folded per partition per chunk
N_CHUNKS = ROWS * C // (128 * FD)   # 16
MM_N = 512                     # matmul moving free dim (one PSUM bank)
TAU = float(np.log(1.5))       # softplus(-ln2) boundary for "easy"

_CACHE = {}


def _pin_act_tables():
    """Force the single LUT set containing both exp and ln so the kernel
    loads one ACT table instead of ping-ponging between two (1.3us/reload).
    Set indices must keep matching act_info.json, so empty the others."""
    import concourse.bacc as bacc
    import concourse.hw_specs as hw_specs

    if getattr(hw_specs, "_act_tables_pinned", False):
        return
    orig = hw_specs.get_activation_tables

    def patched(arch):
        tabs = dict(orig(arch))
        keep = "natural_log_exp_and_others"
        if keep in tabs:
            tabs = {n: (s if n == keep else set()) for n, s in tabs.items()}
        return tabs

    hw_specs._act_tables_pinned = True
    hw_specs.get_activation_tables = patched
    bacc.get_activation_tables = patched


def _build_nc(dt_prod):
    import concourse.bacc as bacc
    import concourse.tile as tile
    from concourse import mybir

    _pin_act_tables()

    f32 = mybir.dt.float32
    AF = mybir.ActivationFunctionType
    OP = mybir.AluOpType

    nc = bacc.Bacc(None)
    pred = nc.dram_tensor("pred", [ROWS, C], f32, kind="ExternalInput")
    targ = nc.dram_tensor("targ", [ROWS, C], f32, kind="ExternalInput")
    out = nc.dram_tensor("partials", [1, 7 * MM_N], f32, kind="ExternalOutput")

    # row = m*FD + p*J + j ; per-partition contiguous J*C elements
    pred_r = pred.rearrange("(m p j) c -> m p (j c)", p=128, j=J)
    targ_r = targ.rearrange("(m p j) c -> m p (j c)", p=128, j=J)

    with tile.TileContext(nc) as tc:
        with (
            tc.tile_pool(name="singles", bufs=1) as singles,
            tc.tile_pool(name="io", bufs=3) as io,
            tc.tile_pool(name="work", bufs=2) as work,
            tc.tile_pool(name="psum", bufs=1, space="PSUM") as psum_pool,
        ):
            ones = singles.tile([128, 1], dt_prod)
            nc.vector.memset(ones, 1.0)
            # bias const for the shifted-Ln trick: ln((2/3)ex + 2/3) = sp - tau
            c23 = singles.tile([128, 1], f32)
            nc.vector.memset(c23, 2.0 / 3.0)
            acc = [
                psum_pool.tile([1, MM_N], f32, tag=f"acc{s}", name=f"acc{s}")
                for s in range(7)
            ]
            # Warmup matmul consumes the ones-memset dependency so that
            # steady-state matmuls carry at most one sync wait (walrus
            # LDWEIGHTS codegen supports only one).
            warm = psum_pool.tile([1, 1], f32, tag="warm")
            nc.tensor.matmul(warm, ones, ones, start=True, stop=True)
            n_sub = FD // MM_N
            for m in range(N_CHUNKS):
                x = io.tile([128, FD], f32, tag="x")
                t = io.tile([128, FD], f32, tag="t")
                nc.sync.dma_start(x, pred_r[m])
                nc.sync.dma_start(t, targ_r[m])

                # bf16 casts of t and t-1 on ACT (ScalarE has capacity)
                tb = io.tile([128, FD], dt_prod, tag="tb")
                tb2 = io.tile([128, FD], dt_prod, tag="tb2")
                nc.scalar.copy(tb, t)
                nc.scalar.activation(tb2, t, AF.Copy, bias=-1.0)

                # softplus via exp/ln (no softplus LUT in this build), with
                # the tau-shift folded into Ln at f32 internal precision:
                # sp2 = ln((2/3)*ex + 2/3) = softplus(x) - tau, so the bf16
                # output keeps the exact f32 easy/hard classification sign.
                ex = work.tile([128, FD], mybir.dt.float32, tag="ex")
                sp2 = work.tile([128, FD], dt_prod, tag="sp2")
                nc.scalar.activation(ex, x, AF.Exp)
                nc.scalar.activation(
                    sp2, ex, AF.Ln, bias=c23[:, 0:1], scale=2.0 / 3.0
                )

                spn2 = work.tile([128, FD], dt_prod, tag="spn2")
                tspn2 = work.tile([128, FD], dt_prod, tag="tspn2")
                tnsp2 = work.tile([128, FD], dt_prod, tag="tnsp2")
                relu1 = work.tile([128, FD], dt_prod, tag="relu1")
                m0 = work.tile([128, FD], dt_prod, tag="m0")
                ge1 = work.tile([128, FD], dt_prod, tag="ge1")
                ge0 = work.tile([128, FD], dt_prod, tag="ge0")
                # spn2 = sp2 - x = softplus(-x) - tau  (x still f32: 1x op)
                nc.vector.scalar_tensor_tensor(
                    spn2, sp2, 0.0, x, OP.bypass, OP.subtract
                )
                nc.vector.tensor_tensor(tspn2, tb, spn2, OP.mult)
                nc.vector.tensor_tensor(tnsp2, tb2, sp2, OP.mult)
                # hard-sample clamps + counts (scalars 0.0 exact in bf16)
                nc.vector.tensor_scalar(relu1, tspn2, 0.0, None, OP.max)
                nc.vector.tensor_scalar(m0, tnsp2, 0.0, None, OP.min)
                nc.vector.tensor_scalar(ge1, tspn2, 0.0, None, OP.is_gt)
                nc.vector.tensor_scalar(ge0, tnsp2, 0.0, None, OP.is_lt)

                movers = [tb, tspn2, tnsp2, relu1, m0, ge1, ge0]
                for jj in range(n_sub):
                    for s, mv in enumerate(movers):
                        nc.tensor.matmul(
                            acc[s][:, :],
                            ones[:, :],
                            mv[:, jj * MM_N : (jj + 1) * MM_N],
                            start=(m == 0 and jj == 0),
                            stop=(m == N_CHUNKS - 1 and jj == n_sub - 1),
                        )

            res = singles.tile([1, 7 * MM_N], f32)
            for s in range(7):
                nc.scalar.copy(res[:, s * MM_N : (s + 1) * MM_N], acc[s][:, :])
            nc.sync.dma_start(out[:, :], res)
    nc.finalize()
    return nc


def _get_nc():
    if "nc" not in _CACHE:
        from concourse import mybir

        _CACHE["nc"] = _build_nc(mybir.dt.bfloat16)
    return _CACHE["nc"]


def _combine(parts):
    """parts: [n_cores, 7, MM_N] raw psum rows -> final scalar loss."""
    # psum column n sums rows with (j % (MM_N//C)) == n//C at col n % C,
    # summed over chunks/jj; fold the leftover j-groups and cores.
    S = parts.reshape(-1, 7, MM_N // C, C).sum(axis=(0, 2), dtype=np.float64)
    pos, t2, u2, r1, nr0, n1, n0 = S
    tau = float(np.float32(TAU))
    B = float(B_TOTAL)
    # t2 = sum t*(spn - tau) ; u2 = sum (t-1)*(sp - tau)
    sb1 = t2 + tau * pos
    sb0 = tau * (B - pos) - u2
    r0 = -nr0
    se1 = sb1 - r1 - tau * n1
    se0 = sb0 - r0 - tau * n0
    B = float(B_TOTAL)
    bal = 0.5 * B
    pos_gt = pos >= bal
    maj_cnt = np.where(pos_gt, pos, B - pos)
    min_cnt = B - maj_cnt
    maj_scale = bal / np.maximum(maj_cnt, 1.0)
    min_scale = np.where(min_cnt > 0, (B - bal) / np.maximum(min_cnt, 1.0), 1.0)
    s_maj_hard = np.where(pos_gt, sb1 - se1, sb0 - se0)
    s_min = np.where(pos_gt, sb0, sb1)
    total = (maj_scale * s_maj_hard + min_scale * s_min).sum()
    return np.float32(total / (B * C))


def kernel(pred: np.ndarray, target: np.ndarray) -> np.ndarray:
    from concourse.bass_utils import run_bass_kernel_spmd

    nc = _get_nc()
    pred = np.ascontiguousarray(pred, dtype=np.float32)
    target = np.ascontiguousarray(target, dtype=np.float32)
    in_maps = [
        {
            "pred": pred[i * ROWS : (i + 1) * ROWS],
            "targ": target[i * ROWS : (i + 1) * ROWS],
        }
        for i in range(N_CORES)
    ]
    res = run_bass_kernel_spmd(nc, in_maps, core_ids=list(range(N_CORES)))
    parts = np.stack([r["partials"].reshape(7, MM_N) for r in res.results])
    return _combine(parts)


# revision 17
# speedup vs baseline: 1.1358x; 1.0703x over previous
"""BalanceLoss Trainium2 kernel.

Math restructuring (see reference _balance_loss):
  - pos_gt = (pos_sum >= B/2), neg_gt = (pos_sum < B/2) are always
    complementary, so every element is in exactly one of maj/min groups and
    maj_cnt/min_cnt = max/min(pos_sum, neg_sum): no 2nd reduction pass.
  - easy <=> (2t-1)*pred > ln2  <=>  (t==1 and softplus(-x) < ln1.5) or
    (t==0 and softplus(x) < ln1.5).
  - loss decomposes into 5 per-column sums over the batch:
        pos_sum = sum t
        S_B1 = sum_{t=1} softplus(-x)            (bce over positives)
        S_E1 = sum_{t=1, easy} softplus(-x)
        S_B0 = sum_{t=0} softplus(x)             (bce over negatives)
        S_E0 = sum_{t=0, easy} softplus(x)
    then per column:
        maj = positives if pos_gt else negatives
        loss_c = maj_scale*(S_Bmaj - S_Emaj) + min_scale*S_Bmin
        total = sum_c loss_c / (B*C)

On device (per core, data-parallel over batch), per chunk:
  ACT: ex = exp(x); sp = ln(ex+1) = softplus(x)    (no softplus LUT in build)
       tb = bf16 copy of t
  DVE: spn = sp - x; tspn = tb*spn; tm1 = tb-1; tnsp = tm1*sp  [= -(1-t)sp]
       relu1 = max(tspn-tau, 0)   m0 = min(tnsp+tau, 0)   (dual-op TS, 4x)
       ge1 = [tspn >= tau]        ge0 = [tnsp <= -tau]    (hard counts)
  PE:  ones[128,1]^T @ {tb,tspn,tnsp,relu1,m0,ge1,ge0} -> 7 PSUM rows
Host: S_E1 = S_B1 - R1 - tau*N1 with R1 = sum(relu1), R0 = -sum(m0), etc.
"""

import numpy as np

B_TOTAL = 131072
C = 128
N_CORES = 8
ROWS = B_TOTAL // N_CORES      # 16384 rows per core
FD = 2048                      # free-dim elements per chunk tile
J = FD // C                    # rows folded per partition per chunk
N_CHUNKS = ROWS * C // (128 * FD)   # 16
MM_N = 512                     # matmul moving free dim (one PSUM bank)
TAU = float(np.log(1.5))       # softplus(-ln2) boundary for "easy"

_CACHE = {}


def _pin_act_tables():
    """Force the single LUT set containing both exp and ln so the kernel
    loads one ACT table instead of ping-ponging between two (1.3us/reload).
    Set indices must keep matching act_info.json, so empty the others."""
    import concourse.bacc as bacc
    import concourse.hw_specs as hw_specs

    if getattr(hw_specs, "_act_tables_pinned", False):
        return
    orig = hw_specs.get_activation_tables

    def patched(arch):
        tabs = dict(orig(arch))
        keep = "natural_log_exp_and_others"
        if keep in tabs:
            tabs = {n: (s if n == keep else set()) for n, s in tabs.items()}
        return tabs

    hw_specs._act_tables_pinned = True
    hw_specs.get_activation_tables = patched
    bacc.get_activation_tables = patched


def _build_nc(dt_prod):
    import concourse.bacc as bacc
    import concourse.tile as tile
    from concourse import mybir

    _pin_act_tables()

    f32 = mybir.dt.float32
    AF = mybir.ActivationFunctionType
    OP = mybir.AluOpType

    nc = bacc.Bacc(None)
    pred = nc.dram_tensor("pred", [ROWS, C], f32, kind="ExternalInput")
    targ = nc.dram_tensor("targ", [ROWS, C], f32, kind="ExternalInput")
    out = nc.dram_tensor("partials", [1, 7 * MM_N], f32, kind="ExternalOutput")

    # row = m*FD + p*J + j ; per-partition contiguous J*C elements
    pred_r = pred.rearrange("(m p j) c -> m p (j c)", p=128, j=J)
    targ_r = targ.rearrange("(m p j) c -> m p (j c)", p=128, j=J)

    with tile.TileContext(nc) as tc:
        with (
            tc.tile_pool(name="singles", bufs=1) as singles,
            tc.tile_pool(name="io", bufs=3) as io,
            tc.tile_pool(name="work", bufs=2) as work,
            tc.tile_pool(name="psum", bufs=1, space="PSUM") as psum_pool,
        ):
            ones = singles.tile([128, 1], dt_prod)
            nc.vector.memset(ones, 1.0)
            # bias const for the shifted-Ln trick: ln((2/3)ex + 2/3) = sp - tau
            c23 = singles.tile([128, 1], f32)
            nc.vector.memset(c23, 2.0 / 3.0)
            acc = [
                psum_pool.tile([1, MM_N], f32, tag=f"acc{s}", name=f"acc{s}")
                for s in range(7)
            ]
            # Warmup matmul consumes the ones-memset dependency so that
            # steady-state matmuls carry at most one sync wait (walrus
            # LDWEIGHTS codegen supports only one).
            warm = psum_pool.tile([1, 1], f32, tag="warm")
            nc.tensor.matmul(warm, ones, ones, start=True, stop=True)
            n_sub = FD // MM_N
            for m in range(N_CHUNKS):
                x = io.tile([128, FD], f32, tag="x")
                t = io.tile([128, FD], f32, tag="t")
                nc.sync.dma_start(x, pred_r[m])
                nc.sync.dma_start(t, targ_r[m])

                # softplus via exp/ln (no softplus LUT in this build), with
                # the tau-shift folded into Ln at f32 internal precision:
                # ln((2/3)*e^v + 2/3) = softplus(v) - tau, so the bf16
                # output keeps the exact f32 easy/hard classification sign.
                # Both streams get their own exp/ln chain on ACT.
                ex = work.tile([128, FD], mybir.dt.float32, tag="ex")
                exn = work.tile([128, FD], mybir.dt.float32, tag="exn")
                sp2 = work.tile([128, FD], dt_prod, tag="sp2")
                spn2 = work.tile([128, FD], dt_prod, tag="spn2")
                nc.scalar.activation(ex, x, AF.Exp)
                nc.scalar.activation(
                    sp2, ex, AF.Ln, bias=c23[:, 0:1], scale=2.0 / 3.0
                )
                nc.scalar.activation(exn, x, AF.Exp, scale=-1.0)
                nc.scalar.activation(
                    spn2, exn, AF.Ln, bias=c23[:, 0:1], scale=2.0 / 3.0
                )

                # bf16 cast of t on DVE (TS single-src f32 runs at 2x)
                tb = io.tile([128, FD], dt_prod, tag="tb")
                tm1 = io.tile([128, FD], dt_prod, tag="tm1")
                nc.vector.tensor_scalar(tb, t, 0.0, None, OP.add)
                nc.vector.tensor_scalar(tm1, tb, 1.0, None, OP.subtract)

                tspn2 = work.tile([128, FD], dt_prod, tag="tspn2")
                tnsp2 = work.tile([128, FD], dt_prod, tag="tnsp2")
                relu1 = work.tile([128, FD], dt_prod, tag="relu1")
                m0 = work.tile([128, FD], dt_prod, tag="m0")
                ge1 = work.tile([128, FD], dt_prod, tag="ge1")
                ge0 = work.tile([128, FD], dt_prod, tag="ge0")
                nc.vector.tensor_tensor(tspn2, tb, spn2, OP.mult)
                nc.vector.tensor_tensor(tnsp2, tm1, sp2, OP.mult)
                # hard-sample clamps + counts (scalars 0.0 exact in bf16)
                nc.vector.tensor_scalar(relu1, tspn2, 0.0, None, OP.max)
                nc.vector.tensor_scalar(m0, tnsp2, 0.0, None, OP.min)
                nc.vector.tensor_scalar(ge1, tspn2, 0.0, None, OP.is_gt)
                nc.vector.tensor_scalar(ge0, tnsp2, 0.0, None, OP.is_lt)

                movers = [tb, tspn2, tnsp2, relu1, m0, ge1, ge0]
                for jj in range(n_sub):
                    for s, mv in enumerate(movers):
                        nc.tensor.matmul(
                            acc[s][:, :],
                            ones[:, :],
                            mv[:, jj * MM_N : (jj + 1) * MM_N],
                            start=(m == 0 and jj == 0),
                            stop=(m == N_CHUNKS - 1 and jj == n_sub - 1),
                        )

            res = singles.tile([1, 7 * MM_N], f32)
            for s in range(7):
                nc.scalar.copy(res[:, s * MM_N : (s + 1) * MM_N], acc[s][:, :])
            nc.sync.dma_start(out[:, :], res)
    nc.finalize()
    return nc


def _get_nc():
    if "nc" not in _CACHE:
        from concourse import mybir

        _CACHE["nc"] = _build_nc(mybir.dt.bfloat16)
    return _CACHE["nc"]


def _combine(parts):
    """parts: [n_cores, 7, MM_N] raw psum rows -> final scalar loss."""
    # psum column n sums rows with (j % (MM_N//C)) == n//C at col n % C,
    # summed over chunks/jj; fold the leftover j-groups and cores.
    S = parts.reshape(-1, 7, MM_N // C, C).sum(axis=(0, 2), dtype=np.float64)
    pos, t2, u2, r1, nr0, n1, n0 = S
    tau = float(np.float32(TAU))
    B = float(B_TOTAL)
    # t2 = sum t*(spn - tau) ; u2 = sum (t-1)*(sp - tau)
    sb1 = t2 + tau * pos
    sb0 = tau * (B - pos) - u2
    r0 = -nr0
    se1 = sb1 - r1 - tau * n1
    se0 = sb0 - r0 - tau * n0
    B = float(B_TOTAL)
    bal = 0.5 * B
    pos_gt = pos >= bal
    maj_cnt = np.where(pos_gt, pos, B - pos)
    min_cnt = B - maj_cnt
    maj_scale = bal / np.maximum(maj_cnt, 1.0)
    min_scale = np.where(min_cnt > 0, (B - bal) / np.maximum(min_cnt, 1.0), 1.0)
    s_maj_hard = np.where(pos_gt, sb1 - se1, sb0 - se0)
    s_min = np.where(pos_gt, sb0, sb1)
    total = (maj_scale * s_maj_hard + min_scale * s_min).sum()
    return np.float32(total / (B * C))


def kernel(pred: np.ndarray, target: np.ndarray) -> np.ndarray:
    from concourse.bass_utils import run_bass_kernel_spmd

    nc = _get_nc()
    pred = np.ascontiguousarray(pred, dtype=np.float32)
    target = np.ascontiguousarray(target, dtype=np.float32)
    in_maps = [
        {
            "pred": pred[i * ROWS : (i + 1) * ROWS],
            "targ": target[i * ROWS : (i + 1) * ROWS],
        }
        for i in range(N_CORES)
    ]
    res = run_bass_kernel_spmd(nc, in_maps, core_ids=list(range(N_CORES)))
    parts = np.stack([r["partials"].reshape(7, MM_N) for r in res.results])
    return _combine(parts)
